# revision 1
# baseline (speedup 1.0000x reference)
"""Trainium2 Bass kernel for nn_Conv2d_68298569941797.

Conv2d: data [32,1,224,224] f32 (x) weight [64,1,3,3] f32 -> out [32,64,222,222] f32
(valid padding, stride 1, cross-correlation).

Strategy (data-parallel over batch, 4 images per NeuronCore x 8 cores):
  The conv is lowered to a single stationary matmul per output chunk.
  Output rows are split into two halves (0..110 / 111..221). The stationary
  operand lhsT is [K=18, M=128]: K = (half, ky, kx), M = (half, out_channel),
  with zeros in the cross-half blocks. The moving operand rhs [18, N] is read
  from 18 shifted copies of the image resident in SBUF: partition
  k = (h, ky, kx) holds the image shifted by (111*h + ky) rows and kx cols.
  One matmul column computes all 128 = 2x64 outputs for one output pixel pair
  ((y, x) for half 0 and (y+111, x) for half 1).

  Chunks: 2 output rows x 222 cols = 444 columns per matmul (fits one PSUM
  bank, and N>=256 keeps float32r matmul at 1 cycle/row). 4 chunks stage into
  one SBUF tile [128, 1776] whose free dim maps to 8 contiguous output rows,
  so the output DMA writes contiguous 7104B runs per (half, channel).

This file is self-contained: shapes/sharding are hardcoded; it only imports
installed packages (numpy, concourse).
"""

import numpy as np

import concourse.bass as bass
import concourse.mybir as mybir
import concourse.tile as tile
from concourse import bacc
from concourse.bass_utils import run_bass_kernel_spmd

N_CORES = 8
B, H, W = 32, 224, 224
O, KH, KW = 64, 3, 3
OH, OW = 222, 222
BPC = B // N_CORES          # images per core
HALF = OH // 2              # 111 output rows per half
KP = 18                     # contraction: (half, ky, kx)
M = 128                     # outputs per column: (half, out_channel)
SEG = 111 * W               # 24864: contiguous elems loaded per partition
IMG = H * W                 # 50176
DATA_LEN = BPC * IMG + 2    # flat padded per-core input (+2: shift-window tail)
OIMG = O * OH * OW          # per-image output elems
CHUNK_ROWS = 2              # output rows per matmul chunk
CHUNK_N = CHUNK_ROWS * OW   # 444 matmul columns
BLK_CHUNKS = 8              # chunks per staged output DMA
BLK_N = BLK_CHUNKS * CHUNK_N
# block base rows: 6 blocks of 16 rows + one final overlapping block
BLOCK_YS = [16 * j for j in range(6)] + [95]

MM_DT = mybir.dt.float32r


def _build_body(tc, data_ap, weight_ap, out_ap, reps=1, variant="full"):
    nc = tc.nc
    data_t = data_ap.tensor
    weight_t = weight_ap.tensor
    out_t = out_ap.tensor
    do_in = variant not in ("noin",)
    do_mm = variant not in ("nocompute", "dmaonly")
    do_out = variant not in ("noout",)

    with (
        tc.tile_pool(name="const", bufs=1) as const_pool,
        tc.tile_pool(name="imgp", bufs=1) as img_pool,
        tc.tile_pool(name="psp", bufs=8, space="PSUM") as psum_pool,
        tc.tile_pool(name="stp", bufs=3) as stage_pool,
    ):
        # lhsT [18, 128]: host-prescattered (see make_in_maps), loaded with a
        # single SWDGE DMA that casts f32 -> f32r (the fast fp32 matmul
        # format; producers of f32r-consumed data must write f32r).
        lhsT = const_pool.tile([KP, M], MM_DT)
        nc.sync.dma_start(lhsT[:], bass.AP(weight_t, 0, [[M, KP], [1, M]]))

        for b in [b for _ in range(reps) for b in range(BPC)]:
            # 18 shifted image copies; partition k=(h,ky,kx) holds the
            # contiguous window data[b].flat[(111h+ky)*224+kx :][:SEG]
            img3 = img_pool.tile([KP, 111, W], MM_DT)
            if do_in:
                # 4 HWDGE loads (ACT ring): [9 partitions, band] each; the
                # 9 shifted copies come from (ky, kx) source dims.
                for h in range(2):
                    for r0, R in ((0, 56), (56, 55)):
                        src = bass.AP(
                            data_t, b * IMG + (HALF * h + r0) * W,
                            [[W, 3], [1, 3], [1, R * W]],
                        )
                        nc.scalar.dma_start(
                            img3[h * 9:(h + 1) * 9, r0:r0 + R, :], src)

            for Y in BLOCK_YS:
                stage = stage_pool.tile([M, BLK_N], mybir.dt.float32)
                if variant == "dmaonly":
                    nc.gpsimd.memset(stage[:], 0)
                if do_mm:
                    for i in range(BLK_CHUNKS):
                        y0 = Y + CHUNK_ROWS * i
                        ps = psum_pool.tile([M, CHUNK_N], mybir.dt.float32)
                        rhs = img3[:, y0:y0 + CHUNK_ROWS, 0:OW]
                        nc.tensor.matmul(
                            ps[:], lhsT[:], rhs,
                            start=True, stop=True,
                        )
                        nc.vector.tensor_copy(
                            stage[:, i * CHUNK_N:(i + 1) * CHUNK_N], ps[:])
                if do_out:
                    # stage free dim = 16 contiguous output rows starting at Y
                    dest = bass.AP(
                        out_t, b * OIMG + Y * OW,
                        [[HALF * OW, 2], [OH * OW, 64], [1, BLK_N]],
                    )
                    nc.sync.dma_start(dest, stage[:])


_NC_CACHE = {}


def _get_nc(reps=1, variant="full"):
    key = (reps, variant)
    if key not in _NC_CACHE:
        nc = bacc.Bacc(
            "TRN2",
            target_bir_lowering=False,
            debug=False,
            num_devices=N_CORES,
        )
        data = nc.dram_tensor(
            "data", [DATA_LEN], MM_DT, kind="ExternalInput").ap()
        weight = nc.dram_tensor(
            "weight", [KP, M], MM_DT,
            kind="ExternalInput").ap()
        out = nc.dram_tensor(
            "out", [BPC, O, OH, OW], mybir.dt.float32,
            kind="ExternalOutput").ap()
        with tile.TileContext(nc) as tc:
            _build_body(tc, data, weight, out, reps=reps, variant=variant)
        nc.compile()
        _NC_CACHE[key] = nc
    return _NC_CACHE[key]


def make_in_maps(data, weight):
    data = np.ascontiguousarray(np.asarray(data), dtype=np.float32)
    weight = np.ascontiguousarray(np.asarray(weight), dtype=np.float32)
    # host-side scatter of w[o,0,ky,kx] into lhsT [K=(h,ky,kx), M=(h,o)]
    lhsT = np.zeros((KP, M), np.float32)
    blk = weight[:, 0].transpose(1, 2, 0).reshape(9, O)  # [(ky,kx), o]
    for h in range(2):
        lhsT[h * 9:(h + 1) * 9, h * O:(h + 1) * O] = blk
    in_maps = []
    for c in range(N_CORES):
        flat = data[c * BPC:(c + 1) * BPC].reshape(-1)
        flat = np.concatenate([flat, np.zeros(2, np.float32)])
        in_maps.append({"data": flat, "weight": lhsT})
    return in_maps


def kernel(data, weight):
    nc = _get_nc()
    res = run_bass_kernel_spmd(
        nc, make_in_maps(data, weight), core_ids=list(range(N_CORES)))
    return np.concatenate([r["out"] for r in res.results], axis=0)



# revision 2
# speedup vs baseline: 1.7526x; 1.7526x over previous
"""Trainium2 Bass kernel for nn_Conv2d_68298569941797.

Conv2d: data [32,1,224,224] f32 (x) weight [64,1,3,3] f32 -> out [32,64,222,222] f32
(valid padding, stride 1, cross-correlation).

Data-parallel over batch: 4 images per NeuronCore x 8 cores. Same matmul
formulation as v1 (stationary lhsT [K=18,M=128], K=(half,ky,kx),
M=(half,out_channel), block-diagonal; rhs columns stream from shifted image
copies resident in SBUF), but the data movement is restructured for DMA
throughput:

  * Input: 6 SWDGE loads per image fill imgall [128, 40, 224]: partition
    32*j + 9*h + (ky,kx) holds 40 contiguous image rows starting at
    (111*h + 38*j + ky) shifted by kx, for block j of 38 output rows at
    Y=38j. The gpsimd (SWDGE) ring never queues behind output, and bufs=2
    overlaps image b+1's load with image b's compute. Matmul operand base
    partitions must be in {0,32,64}, hence blocks at those bases and lhsT
    replicated at the same three bases.
  * Output: staged in [128, 56*222] tiles (56 output rows x both halves) so
    each out-DMA descriptor is a 49.7KB contiguous HBM run, consecutive stage
    DMAs alternate between the two HWDGE rings (nc.sync / nc.scalar), and --
    critically -- the dest AP's OUTER dim is the 64 channels: DMA descriptors
    are sprayed across the 16 SDMA engines by the slowest AP dim, so an outer
    dim of 2 (v1: halves) used ~2 engines (~56 GB/s); outer dim 64 with stage
    partition p = 2*o + h uses all 16 (~5x faster end-to-end).
  * PSUM->SBUF chunk copies stay on DVE ([128,444] each, 8 PSUM banks).

Per image: stage0 = output rows 0..55 (chunks y=0,2..54), stage1 = rows
56..110 (chunks y=56,58..108 plus a final 1-row-overlap chunk y=109). Chunk y
maps to input block j=y//38, local row ly=y%38 (every chunk's two rows + 2
tap rows live inside one block's 40 stored rows).

Self-contained: only imports numpy + concourse.
"""

import numpy as np

import concourse.bass as bass
import concourse.mybir as mybir
import concourse.tile as tile
from concourse import bacc
from concourse.bass_utils import run_bass_kernel_spmd

N_CORES = 8
B, H, W = 32, 224, 224
O, KH, KW = 64, 3, 3
OH, OW = 222, 222
BPC = B // N_CORES          # images per core
HALF = OH // 2              # 111 output rows per half
KP = 18                     # contraction: (half, ky, kx)
M = 128                     # outputs per column: (half, out_channel)
IMG = H * W                 # 50176
OIMG = O * OH * OW          # per-image output elems
NBLK = 3                    # input blocks per half, at Y=38j
BLK_ROWS = 38               # output rows per block
IMG_ROWS = 40               # rows stored per imgall partition (38 + 2 taps)
WP = NBLK * 32 - 32 + KP    # 82 weight partitions (3 lhsT copies)
# worst-case flat-read overrun past the last image (j=2,h=1,ky=2,kx=2)
PAD = (BLK_ROWS * 2 + IMG_ROWS + HALF + 2 - H) * W + 2  # 1122
DATA_LEN = BPC * IMG + PAD
CHUNK_N = 2 * OW            # 444 matmul columns (2 output rows)
# (y0, nrows, chunk y values) per stage
STAGES = [
    (0, 56, list(range(0, 56, 2))),
    (56, 55, list(range(56, 109, 2)) + [109]),
]

MM_DT = mybir.dt.float32r


def _build_body(tc, data_ap, weight_ap, out_ap, reps=1, variant="full"):
    nc = tc.nc
    data_t = data_ap.tensor
    weight_t = weight_ap.tensor
    out_t = out_ap.tensor
    do_in = variant not in ("noin",)
    do_mm = variant not in ("nocompute", "dmaonly")
    do_out = variant not in ("noout",)

    with (
        tc.tile_pool(name="const", bufs=1) as const_pool,
        tc.tile_pool(name="imgp", bufs=2) as img_pool,
        tc.tile_pool(name="psp", bufs=8, space="PSUM") as psum_pool,
        tc.tile_pool(name="stp", bufs=2) as stage_pool,
    ):
        # lhsT replicated at base partitions 0/32/64/96 (rows 18..31 etc of
        # the dram tensor are zero padding)
        lhsT = const_pool.tile([WP, M], MM_DT)
        nc.sync.dma_start(lhsT[:], bass.AP(weight_t, 0, [[M, WP], [1, M]]))

        out_ring = 0
        for b in [b for _ in range(reps) for b in range(BPC)]:
            # 3 loads fill the shifted copies for blocks j=0..2 of image b
            imgall = img_pool.tile([128, IMG_ROWS, W], MM_DT)
            if do_in:
                # DMA APs allow at most 3 dims, so one load per (blk, half)
                for j in range(NBLK):
                    for h in range(2):
                        src = bass.AP(
                            data_t,
                            b * IMG + (BLK_ROWS * j + HALF * h) * W,
                            [[W, 3], [1, 3], [1, IMG_ROWS * W]],
                        )
                        nc.gpsimd.dma_start(
                            imgall[32 * j + 9 * h:32 * j + 9 * h + 9, :, :],
                            src)

            for y0, nrows, ys in STAGES:
                stage = stage_pool.tile([M, nrows * OW], mybir.dt.float32)
                if variant == "dmaonly":
                    nc.gpsimd.memset(stage[:], 0)
                if do_mm:
                    for y in ys:
                        j, ly = y // BLK_ROWS, y % BLK_ROWS
                        ps = psum_pool.tile([M, CHUNK_N], mybir.dt.float32)
                        rhs = imgall[32 * j:32 * j + KP, ly:ly + 2, 0:OW]
                        nc.tensor.matmul(
                            ps[:], lhsT[32 * j:32 * j + KP, :], rhs,
                            start=True, stop=True)
                        r = y - y0
                        nc.vector.tensor_copy(
                            stage[:, r * OW:(r + 2) * OW], ps[:])
                if do_out:
                    # outer dim = 64 channels: DMA descriptors spray across
                    # SDMA engines by the slowest dim, so it must be large
                    # (outer=2 halves measured ~56 GB/s; this is the fix for
                    # that). Stage partition p = 2*o + h matches (o, h) order.
                    dest = bass.AP(
                        out_t, b * OIMG + y0 * OW,
                        [[OH * OW, O], [HALF * OW, 2], [1, nrows * OW]],
                    )
                    eng = nc.sync if out_ring % 2 == 0 else nc.scalar
                    eng.dma_start(dest, stage[:])
                    out_ring += 1


_NC_CACHE = {}


def _get_nc(reps=1, variant="full"):
    key = (reps, variant)
    if key not in _NC_CACHE:
        nc = bacc.Bacc(
            "TRN2",
            target_bir_lowering=False,
            debug=False,
            num_devices=N_CORES,
        )
        data = nc.dram_tensor(
            "data", [DATA_LEN], MM_DT, kind="ExternalInput").ap()
        weight = nc.dram_tensor(
            "weight", [WP, M], MM_DT,
            kind="ExternalInput").ap()
        out = nc.dram_tensor(
            "out", [BPC, O, OH, OW], mybir.dt.float32,
            kind="ExternalOutput").ap()
        with tile.TileContext(nc) as tc:
            _build_body(tc, data, weight, out, reps=reps, variant=variant)
        nc.compile()
        _NC_CACHE[key] = nc
    return _NC_CACHE[key]


def make_in_maps(data, weight):
    data = np.ascontiguousarray(np.asarray(data), dtype=np.float32)
    weight = np.ascontiguousarray(np.asarray(weight), dtype=np.float32)
    # host-side scatter of w[o,0,ky,kx] into lhsT [K=(h,ky,kx), M=(o,h)]
    # (column m = 2*o + h so stage partitions are channel-major for the
    # out-DMA spray), replicated at partition bases 0/32/64
    lhsT = np.zeros((WP, M), np.float32)
    blk = weight[:, 0].transpose(1, 2, 0).reshape(9, O)  # [(ky,kx), o]
    for j in range(NBLK):
        for h in range(2):
            lhsT[32 * j + h * 9:32 * j + (h + 1) * 9, h::2] = blk
    in_maps = []
    for c in range(N_CORES):
        flat = data[c * BPC:(c + 1) * BPC].reshape(-1)
        flat = np.concatenate([flat, np.zeros(PAD, np.float32)])
        in_maps.append({"data": flat, "weight": lhsT})
    return in_maps


def kernel(data, weight):
    nc = _get_nc()
    res = run_bass_kernel_spmd(
        nc, make_in_maps(data, weight), core_ids=list(range(N_CORES)))
    return np.concatenate([r["out"] for r in res.results], axis=0)
